# revision 10
# baseline (speedup 1.0000x reference)
"""Multi-head attention (B=2, L=2048, D=1024, H=16) on 8 trn2 NeuronCores.

Sharding: tensor-parallel over heads — 2 heads per core. Each core computes
q/k/v projections for its 2 heads, the attention for those heads, and a
row-parallel partial of the output projection (transposed). The host sums
the 8 bf16 partials (the "all-reduce") and adds the biases that were folded
out of the device kernel (bv folded through Wo, plus bo).

Device layout (everything transposed, feature-major):
  xt   [128, KC, R]  : X.T, chunked along the contraction dim
  qt/kt/vt [128, R]  : projections, partitions = 2 heads x 64 head-dims
  va[h][128, NRT,128]: per k-row-tile blocks [v_h | ones] / [ones | v_h];
                       the ones columns make the PV matmul also produce the
                       softmax denominator in the complementary half.
  logitsT [k, q]     : exp() needs no max-subtraction (logits ~ N(0, 0.33^2))

All-bf16 matmul inputs: fp8(+DoubleRow) was measured at rel-err 0.019-0.028
on the final output — over the 2e-2 budget — so it is not used.

Schedule: attention runs in 512-column q units (8 of them). Per k-tile the
two heads' logits land in one [128, 2, 512] PSUM tile (2 banks, bufs=2)
written by a row-group-concurrent MM pair, and ONE ACTIVATE exps both
heads; PV lags by 2 k-tiles so the in-order PE never waits on ACT. ScalarE
(exp) is the pacing engine. The softmax reciprocal is computed as
exp(-ln(d)) so every ACT call lives in the single
natural_log_exp_and_others table set — a raw Reciprocal costs two table
swaps per unit, each ~1.3us plus a >3.4us PE idle that re-throttles the
HAM clock gate to 1.2 GHz.
"""

import numpy as np
import ml_dtypes

import concourse.bass as bass
import concourse.mybir as mybir
import concourse.tile as tile
from concourse import bacc
from concourse.bass_utils import run_bass_kernel_spmd
from concourse.masks import make_identity

B, L, D, H = 2, 2048, 1024, 16
HD = D // H              # 64 head dim
N_CORES = 8
HPC = H // N_CORES       # 2 heads per core
DK = HPC * HD            # 128 local qkv feature dim
R = B * L                # 4096 rows
KC = D // 128            # 8 contraction chunks for the projections
UW = 512                 # attention unit width (q columns)
NUB = L // UW            # 4 units per batch
NRH = R // UW            # 8 projection row-chunk halves
NKT = L // 128           # 16 k tiles per batch
NRT = R // 128           # 32 row tiles
LAG = 2                  # PV k-tile lag behind logits/exp
SCALE = HD ** -0.5

BF16 = mybir.dt.bfloat16
F32 = mybir.dt.float32
Act = mybir.ActivationFunctionType
Alu = mybir.AluOpType

_BF16_NP = ml_dtypes.bfloat16


def _body(tc, nc, xt_d, wqt_d, wkt_d, wvt_d, bq_d, bk_d, wot_d, out_d):
    with (
        tc.tile_pool(name="consts", bufs=1) as constp,
        tc.tile_pool(name="bigs", bufs=1) as bigs,
        tc.tile_pool(name="epool", bufs=4) as epool,
        tc.tile_pool(name="work", bufs=1) as work,
        tc.tile_pool(name="outst", bufs=4) as outst,
    ):
        # ---- load weights / biases ----
        wq_sb = constp.tile([128, KC, DK], BF16)
        wk_sb = constp.tile([128, KC, DK], BF16)
        wv_sb = constp.tile([128, KC, DK], BF16)
        wot_sb = constp.tile([DK, D], BF16)
        bq_sb = constp.tile([DK, 1], F32)
        bk_sb = constp.tile([DK, 1], F32)
        ident = constp.tile([128, 128], BF16)
        nc.sync.dma_start(out=wq_sb, in_=wqt_d[:])
        nc.sync.dma_start(out=wk_sb, in_=wkt_d[:])
        nc.sync.dma_start(out=wv_sb, in_=wvt_d[:])
        nc.sync.dma_start(out=wot_sb, in_=wot_d[:])
        nc.sync.dma_start(out=bq_sb, in_=bq_d[:])
        nc.sync.dma_start(out=bk_sb, in_=bk_d[:])
        make_identity(nc, ident)

        # ---- load X.T (chunked so the projections chase the DMA) ----
        xt_sb = bigs.tile([128, KC, R], BF16)
        for c in range(KC):
            nc.sync.dma_start(out=xt_sb[:, c, :], in_=xt_d[:, c, :])

        qt = bigs.tile([DK, R], BF16)
        kt = bigs.tile([DK, R], BF16)
        vt = bigs.tile([DK, R], BF16)
        yt = bigs.tile([DK, R], BF16)
        va = [bigs.tile([128, NRT, 128], BF16, name=f"va{h}") for h in range(HPC)]
        for h in range(HPC):
            nc.gpsimd.memset(va[h][:], 1.0)

        # ---- q/k/v projections ----
        with tc.tile_pool(name="projpsum", bufs=1, space="PSUM") as projp:
            for wsb, bsb, dest in (
                (wk_sb, bk_sb, kt),
                (wq_sb, bq_sb, qt),
                (wv_sb, None, vt),
            ):
                ps = [
                    projp.tile([128, UW], F32, tag="proj", bufs=NRH, name=f"pp{i}")
                    for i in range(NRH)
                ]
                for c in range(KC):
                    for i in range(NRH):
                        nc.tensor.matmul(
                            ps[i],
                            lhsT=wsb[:, c, :],
                            rhs=xt_sb[:, c, i * UW : (i + 1) * UW],
                            start=(c == 0),
                            stop=(c == KC - 1),
                        )
                for i in range(NRH):
                    cols = slice(i * UW, (i + 1) * UW)
                    if bsb is not None:
                        nc.vector.tensor_scalar_add(
                            out=dest[:, cols], in0=ps[i], scalar1=bsb
                        )
                    else:
                        nc.vector.tensor_copy(out=dest[:, cols], in_=ps[i])

        psum_cm = tc.tile_pool(name="psum", bufs=1, space="PSUM")
        psum = psum_cm.__enter__()

        # ---- va via PE transpose of vt ----
        for t in range(NRT):
            pt = psum.tile([128, 128], BF16, tag="spare", bufs=2, name="pt")
            nc.tensor.transpose(pt, vt[:, t * 128 : (t + 1) * 128], ident)
            for h in range(HPC):
                nc.vector.tensor_copy(
                    out=va[h][:, t, h * HD : (h + 1) * HD],
                    in_=pt[:, h * HD : (h + 1) * HD],
                )

        # ---- attention ----
        def emit_outproj(rc, ofbs):
            # partial outT[ofb-block, unit-cols] = WoTlocal_blk.T @ YT_unit
            for ofb in ofbs:
                po = psum.tile([128, UW], F32, tag="spare", bufs=2, name="po")
                nc.tensor.matmul(
                    po,
                    lhsT=wot_sb[:, ofb * 128 : (ofb + 1) * 128],
                    rhs=yt[:, rc * UW : (rc + 1) * UW],
                    start=True,
                    stop=True,
                )
                ost = outst.tile([128, UW], BF16, name="ost")
                nc.vector.tensor_copy(out=ost, in_=po)
                nc.sync.dma_start(
                    out=out_d[ofb * 128 : (ofb + 1) * 128, rc * UW : (rc + 1) * UW],
                    in_=ost,
                )

        pending_rc = None
        for b in range(B):
            for u in range(NUB):
                qcols = slice(b * L + u * UW, b * L + (u + 1) * UW)
                pv0 = psum.tile([128, UW], F32, tag="pv", bufs=2, name="pv0")
                pv1 = psum.tile([128, UW], F32, tag="pv", bufs=2, name="pv1")
                # software pipeline: PV lags logits/exp by LAG k-tiles so the
                # in-order PE only ever waits on semaphores already satisfied.
                es = {}
                for k in range(NKT + LAG):
                    if k < NKT:
                        kcols = slice(b * L + k * 128, b * L + (k + 1) * 128)
                        # both heads' logits in one 2-bank psum tile; the MM
                        # pair targets disjoint PE row groups (auto
                        # tile_position from base_partition 0 / 64).
                        pl = psum.tile([128, HPC, UW], F32, tag="pl", bufs=2, name="pl")
                        nc.tensor.matmul(
                            pl[:, 0, :], lhsT=kt[0:HD, kcols], rhs=qt[0:HD, qcols],
                            start=True, stop=True,
                        )
                        nc.tensor.matmul(
                            pl[:, 1, :], lhsT=kt[HD:DK, kcols], rhs=qt[HD:DK, qcols],
                            start=True, stop=True,
                        )
                        e = epool.tile([128, HPC, UW], BF16, name="e")
                        nc.scalar.activation(
                            out=e[:, :, :], in_=pl[:, :, :], func=Act.Exp, scale=SCALE
                        )
                        es[k] = e
                    if k >= LAG:
                        j = k - LAG
                        tg = b * NKT + j
                        ep = es.pop(j)
                        nc.tensor.matmul(
                            pv0, lhsT=va[0][:, tg, :], rhs=ep[:, 0, :],
                            start=(j == 0), stop=(j == NKT - 1),
                        )
                        nc.tensor.matmul(
                            pv1, lhsT=va[1][:, tg, :], rhs=ep[:, 1, :],
                            start=(j == 0), stop=(j == NKT - 1),
                        )
                    # previous unit's out-projection, spread 2 blocks/ktile
                    if pending_rc is not None and 2 <= k <= 5:
                        emit_outproj(pending_rc, range((k - 2) * 2, (k - 1) * 2))
                        if k == 5:
                            pending_rc = None
                # pv0 = [Yun_h0 (p 0:64); denom_h0 (p 64:128)]
                # pv1 = [denom_h1 (p 0:64); Yun_h1 (p 64:128)]
                # reciprocal as exp(-ln(d)): stays in the exp table set.
                rsw = work.tile([128, UW], F32, tag="rsw", bufs=2, name="rsw")
                nc.scalar.activation(out=rsw[HD:128, :], in_=pv0[HD:128, :], func=Act.Ln)
                nc.scalar.activation(out=rsw[0:HD, :], in_=pv1[0:HD, :], func=Act.Ln)
                # swap halves across partitions (DMA is the cross-lane engine)
                rr = work.tile([128, UW], F32, tag="rr", bufs=2, name="rr")
                nc.sync.dma_start(out=rr[0:HD, :], in_=rsw[HD:128, :])
                nc.sync.dma_start(out=rr[HD:128, :], in_=rsw[0:HD, :])
                rre = work.tile([128, UW], F32, tag="rre", bufs=2, name="rre")
                nc.scalar.activation(out=rre, in_=rr, func=Act.Exp, scale=-1.0)
                nc.vector.tensor_mul(
                    out=yt[0:HD, qcols], in0=pv0[0:HD, :], in1=rre[0:HD, :]
                )
                nc.vector.tensor_mul(
                    out=yt[HD:DK, qcols], in0=pv1[HD:DK, :], in1=rre[HD:DK, :]
                )
                pending_rc = b * NUB + u

        # ---- last unit's out-projection ----
        emit_outproj(pending_rc, range(D // 128))
        psum_cm.__exit__(None, None, None)


def build_bass():
    nc = bacc.Bacc("TRN2", target_bir_lowering=False, debug=False)
    xt_d = nc.dram_tensor("xt", [128, KC, R], BF16, kind="ExternalInput")
    wqt_d = nc.dram_tensor("wqt", [128, KC, DK], BF16, kind="ExternalInput")
    wkt_d = nc.dram_tensor("wkt", [128, KC, DK], BF16, kind="ExternalInput")
    wvt_d = nc.dram_tensor("wvt", [128, KC, DK], BF16, kind="ExternalInput")
    bq_d = nc.dram_tensor("bq", [DK, 1], F32, kind="ExternalInput")
    bk_d = nc.dram_tensor("bk", [DK, 1], F32, kind="ExternalInput")
    wot_d = nc.dram_tensor("wot", [DK, D], BF16, kind="ExternalInput")
    out_d = nc.dram_tensor("out", [D, R], BF16, kind="ExternalOutput")
    with tile.TileContext(nc) as tc:
        _body(tc, nc, xt_d, wqt_d, wkt_d, wvt_d, bq_d, bk_d, wot_d, out_d)
    nc.compile()
    return nc


_NC = None


def _get_nc():
    global _NC
    if _NC is None:
        _NC = build_bass()
    return _NC


def prepare(inputs):
    """Full inputs -> (per-core in_maps, host-side bias constant)."""
    q = np.asarray(inputs["query"], np.float32)
    Wq = np.asarray(inputs["Wq"], np.float32)
    Wk = np.asarray(inputs["Wk"], np.float32)
    Wv = np.asarray(inputs["Wv"], np.float32)
    Wo = np.asarray(inputs["Wo"], np.float32)
    bq = np.asarray(inputs["bq"], np.float32)
    bk = np.asarray(inputs["bk"], np.float32)
    bv = np.asarray(inputs["bv"], np.float32)
    bo = np.asarray(inputs["bo"], np.float32)

    X = q.reshape(R, D)
    # [p, chunk, r] with in-feature = chunk*128 + p
    xt = np.ascontiguousarray(
        X.T.reshape(KC, 128, R).transpose(1, 0, 2)
    ).astype(_BF16_NP)

    def wslice(W, hs):
        # W[hs].T laid out [p, chunk, m]: in-feat within chunk, chunk, out-feat
        return np.ascontiguousarray(
            W[hs, :].T.reshape(KC, 128, DK).transpose(1, 0, 2)
        ).astype(_BF16_NP)

    in_maps = []
    const = bo.astype(np.float64).copy()
    for c in range(N_CORES):
        hs = slice(c * DK, (c + 1) * DK)
        const += Wo[:, hs].astype(np.float64) @ bv[hs].astype(np.float64)
        in_maps.append(
            {
                "xt": xt,
                "wqt": wslice(Wq, hs),
                "wkt": wslice(Wk, hs),
                "wvt": wslice(Wv, hs),
                "bq": np.ascontiguousarray(bq[hs].reshape(DK, 1)),
                "bk": np.ascontiguousarray(bk[hs].reshape(DK, 1)),
                "wot": np.ascontiguousarray(Wo[:, hs].T).astype(_BF16_NP),
            }
        )
    return in_maps, const


def finish(results, const):
    acc = np.zeros((D, R), np.float64)
    for r in results:
        acc += np.asarray(r["out"], np.float64)
    out = acc.T + const[None, :]
    return out.astype(np.float32).reshape(B, L, D)


def run(in_maps, trace=False, **kwargs):
    nc = _get_nc()
    return run_bass_kernel_spmd(nc, in_maps, list(range(N_CORES)), trace=trace, **kwargs)


def kernel(**inputs):
    in_maps, const = prepare(inputs)
    res = run(in_maps)
    return finish(res.results, const)


# revision 12
# speedup vs baseline: 1.0838x; 1.0838x over previous
"""Multi-head attention (B=2, L=2048, D=1024, H=16) on 8 trn2 NeuronCores.

Sharding: tensor-parallel over heads — 2 heads per core. Each core computes
q/k/v projections for its 2 heads, the attention for those heads, and a
row-parallel partial of the output projection (transposed). The host sums
the 8 bf16 partials (the "all-reduce") and adds the biases that were folded
out of the device kernel (bv folded through Wo, plus bo).

Device layout (everything transposed, feature-major):
  xt   [128, KC, R]  : X.T, chunked along the contraction dim
  qt/kt/vt [128, R]  : projections, partitions = 2 heads x 64 head-dims
  va[h][128, NRT,128]: per k-row-tile blocks [v_h | ones] / [ones | v_h];
                       the ones columns make the PV matmul also produce the
                       softmax denominator in the complementary half.
  logitsT [k, q]     : exp() needs no max-subtraction (logits ~ N(0, 0.33^2))

All-bf16 matmul inputs: fp8(+DoubleRow) was measured at rel-err 0.019-0.028
on the final output — over the 2e-2 budget — so it is not used.

Schedule: attention runs in 512-column q units (8 of them). Per k-tile the
two heads' logits land in one [128, 2, 512] PSUM tile (2 banks, bufs=2)
written by a row-group-concurrent MM pair, and ONE ACTIVATE exps both
heads; PV lags by 2 k-tiles so the in-order PE never waits on ACT. ScalarE
(exp) is the pacing engine. The softmax reciprocal is computed as
exp(-ln(d)) so every ACT call lives in the single
natural_log_exp_and_others table set — a raw Reciprocal costs two table
swaps per unit, each ~1.3us plus a >3.4us PE idle that re-throttles the
HAM clock gate to 1.2 GHz.
"""

import numpy as np
import ml_dtypes

import concourse.bass as bass
import concourse.mybir as mybir
import concourse.tile as tile
from concourse import bacc
from concourse.bass_utils import run_bass_kernel_spmd
from concourse.masks import make_identity

# Constrain the ACT table-set chooser to the one set containing BOTH Exp and
# Ln. Left alone, bacc assigns Exp -> exp_and_others and Ln -> natural_log
# (first set containing each func), which costs 2 table reloads per
# attention unit (~1.3us each) plus a >3.4us ScalarE/PE stall that
# re-throttles the PE HAM clock gate to 1.2 GHz for ~3.4us every unit.
# Set IDs are positional indices into act_info.json, so the dict must keep
# all entries (emptied, not removed).
_orig_get_act_tables = bacc.get_activation_tables


def _ln_exp_only_tables(arch):
    t = _orig_get_act_tables(arch)
    return {
        k: (v if k == "natural_log_exp_and_others" else set())
        for k, v in t.items()
    }


bacc.get_activation_tables = _ln_exp_only_tables

B, L, D, H = 2, 2048, 1024, 16
HD = D // H              # 64 head dim
N_CORES = 8
HPC = H // N_CORES       # 2 heads per core
DK = HPC * HD            # 128 local qkv feature dim
R = B * L                # 4096 rows
KC = D // 128            # 8 contraction chunks for the projections
UW = 512                 # attention unit width (q columns)
NUB = L // UW            # 4 units per batch
NRH = R // UW            # 8 projection row-chunk halves
NKT = L // 128           # 16 k tiles per batch
NRT = R // 128           # 32 row tiles
LAG = 2                  # PV k-tile lag behind logits/exp
SCALE = HD ** -0.5

BF16 = mybir.dt.bfloat16
F32 = mybir.dt.float32
Act = mybir.ActivationFunctionType
Alu = mybir.AluOpType

_BF16_NP = ml_dtypes.bfloat16


def _body(tc, nc, xt_d, wqt_d, wkt_d, wvt_d, bq_d, bk_d, wot_d, out_d):
    with (
        tc.tile_pool(name="consts", bufs=1) as constp,
        tc.tile_pool(name="bigs", bufs=1) as bigs,
        tc.tile_pool(name="epool", bufs=4) as epool,
        tc.tile_pool(name="work", bufs=1) as work,
        tc.tile_pool(name="outst", bufs=4) as outst,
    ):
        # ---- load weights / biases ----
        wq_sb = constp.tile([128, KC, DK], BF16)
        wk_sb = constp.tile([128, KC, DK], BF16)
        wv_sb = constp.tile([128, KC, DK], BF16)
        wot_sb = constp.tile([DK, D], BF16)
        bq_sb = constp.tile([DK, 1], F32)
        bk_sb = constp.tile([DK, 1], F32)
        ident = constp.tile([128, 128], BF16)
        nc.sync.dma_start(out=wq_sb, in_=wqt_d[:])
        nc.sync.dma_start(out=wk_sb, in_=wkt_d[:])
        nc.sync.dma_start(out=wv_sb, in_=wvt_d[:])
        nc.sync.dma_start(out=wot_sb, in_=wot_d[:])
        nc.sync.dma_start(out=bq_sb, in_=bq_d[:])
        nc.sync.dma_start(out=bk_sb, in_=bk_d[:])
        make_identity(nc, ident)

        # ---- load X.T (chunked so the projections chase the DMA) ----
        xt_sb = bigs.tile([128, KC, R], BF16)
        for c in range(KC):
            nc.sync.dma_start(out=xt_sb[:, c, :], in_=xt_d[:, c, :])

        qt = bigs.tile([DK, R], BF16)
        kt = bigs.tile([DK, R], BF16)
        vt = bigs.tile([DK, R], BF16)
        yt = bigs.tile([DK, R], BF16)
        va = [bigs.tile([128, NRT, 128], BF16, name=f"va{h}") for h in range(HPC)]
        for h in range(HPC):
            nc.gpsimd.memset(va[h][:], 1.0)

        # ---- q/k/v projections ----
        with tc.tile_pool(name="projpsum", bufs=1, space="PSUM") as projp:
            for wsb, bsb, dest in (
                (wk_sb, bk_sb, kt),
                (wq_sb, bq_sb, qt),
                (wv_sb, None, vt),
            ):
                ps = [
                    projp.tile([128, UW], F32, tag="proj", bufs=NRH, name=f"pp{i}")
                    for i in range(NRH)
                ]
                for c in range(KC):
                    for i in range(NRH):
                        nc.tensor.matmul(
                            ps[i],
                            lhsT=wsb[:, c, :],
                            rhs=xt_sb[:, c, i * UW : (i + 1) * UW],
                            start=(c == 0),
                            stop=(c == KC - 1),
                        )
                for i in range(NRH):
                    cols = slice(i * UW, (i + 1) * UW)
                    if bsb is not None:
                        nc.vector.tensor_scalar_add(
                            out=dest[:, cols], in0=ps[i], scalar1=bsb
                        )
                    else:
                        nc.vector.tensor_copy(out=dest[:, cols], in_=ps[i])

        psum_cm = tc.tile_pool(name="psum", bufs=1, space="PSUM")
        psum = psum_cm.__enter__()

        # ---- va via PE transpose of vt ----
        for t in range(NRT):
            pt = psum.tile([128, 128], BF16, tag="spare", bufs=2, name="pt")
            nc.tensor.transpose(pt, vt[:, t * 128 : (t + 1) * 128], ident)
            for h in range(HPC):
                nc.vector.tensor_copy(
                    out=va[h][:, t, h * HD : (h + 1) * HD],
                    in_=pt[:, h * HD : (h + 1) * HD],
                )

        # ---- attention ----
        def emit_outproj(rc, ofbs):
            # partial outT[ofb-block, unit-cols] = WoTlocal_blk.T @ YT_unit
            for ofb in ofbs:
                po = psum.tile([128, UW], F32, tag="spare", bufs=2, name="po")
                nc.tensor.matmul(
                    po,
                    lhsT=wot_sb[:, ofb * 128 : (ofb + 1) * 128],
                    rhs=yt[:, rc * UW : (rc + 1) * UW],
                    start=True,
                    stop=True,
                )
                ost = outst.tile([128, UW], BF16, name="ost")
                nc.vector.tensor_copy(out=ost, in_=po)
                nc.sync.dma_start(
                    out=out_d[ofb * 128 : (ofb + 1) * 128, rc * UW : (rc + 1) * UW],
                    in_=ost,
                )

        pending_rc = None
        for b in range(B):
            for u in range(NUB):
                qcols = slice(b * L + u * UW, b * L + (u + 1) * UW)
                pv0 = psum.tile([128, UW], F32, tag="pv", bufs=2, name="pv0")
                pv1 = psum.tile([128, UW], F32, tag="pv", bufs=2, name="pv1")
                # software pipeline: PV lags logits/exp by LAG k-tiles so the
                # in-order PE only ever waits on semaphores already satisfied.
                es = {}
                for k in range(NKT + LAG):
                    if k < NKT:
                        kcols = slice(b * L + k * 128, b * L + (k + 1) * 128)
                        # both heads' logits in one 2-bank psum tile; the MM
                        # pair targets disjoint PE row groups (auto
                        # tile_position from base_partition 0 / 64).
                        pl = psum.tile([128, HPC, UW], F32, tag="pl", bufs=2, name="pl")
                        nc.tensor.matmul(
                            pl[:, 0, :], lhsT=kt[0:HD, kcols], rhs=qt[0:HD, qcols],
                            start=True, stop=True,
                        )
                        nc.tensor.matmul(
                            pl[:, 1, :], lhsT=kt[HD:DK, kcols], rhs=qt[HD:DK, qcols],
                            start=True, stop=True,
                        )
                        e = epool.tile([128, HPC, UW], BF16, name="e")
                        nc.scalar.activation(
                            out=e[:, :, :], in_=pl[:, :, :], func=Act.Exp, scale=SCALE
                        )
                        es[k] = e
                    if k >= LAG:
                        j = k - LAG
                        tg = b * NKT + j
                        ep = es.pop(j)
                        nc.tensor.matmul(
                            pv0, lhsT=va[0][:, tg, :], rhs=ep[:, 0, :],
                            start=(j == 0), stop=(j == NKT - 1),
                        )
                        nc.tensor.matmul(
                            pv1, lhsT=va[1][:, tg, :], rhs=ep[:, 1, :],
                            start=(j == 0), stop=(j == NKT - 1),
                        )
                    # previous unit's out-projection, spread 1 block/ktile
                    if pending_rc is not None and 2 <= k <= 9:
                        emit_outproj(pending_rc, [k - 2])
                        if k == 9:
                            pending_rc = None
                # pv0 = [Yun_h0 (p 0:64); denom_h0 (p 64:128)]
                # pv1 = [denom_h1 (p 0:64); Yun_h1 (p 64:128)]
                # reciprocal as exp(-ln(d)): stays in the exp table set.
                rsw = work.tile([128, UW], F32, tag="rsw", bufs=2, name="rsw")
                nc.scalar.activation(out=rsw[HD:128, :], in_=pv0[HD:128, :], func=Act.Ln)
                nc.scalar.activation(out=rsw[0:HD, :], in_=pv1[0:HD, :], func=Act.Ln)
                # swap halves across partitions (DMA is the cross-lane engine)
                rr = work.tile([128, UW], F32, tag="rr", bufs=2, name="rr")
                nc.sync.dma_start(out=rr[0:HD, :], in_=rsw[HD:128, :])
                nc.sync.dma_start(out=rr[HD:128, :], in_=rsw[0:HD, :])
                rre = work.tile([128, UW], F32, tag="rre", bufs=2, name="rre")
                nc.scalar.activation(out=rre, in_=rr, func=Act.Exp, scale=-1.0)
                nc.vector.tensor_mul(
                    out=yt[0:HD, qcols], in0=pv0[0:HD, :], in1=rre[0:HD, :]
                )
                nc.vector.tensor_mul(
                    out=yt[HD:DK, qcols], in0=pv1[HD:DK, :], in1=rre[HD:DK, :]
                )
                pending_rc = b * NUB + u

        # ---- last unit's out-projection ----
        emit_outproj(pending_rc, range(D // 128))
        psum_cm.__exit__(None, None, None)


def build_bass():
    nc = bacc.Bacc("TRN2", target_bir_lowering=False, debug=False)
    xt_d = nc.dram_tensor("xt", [128, KC, R], BF16, kind="ExternalInput")
    wqt_d = nc.dram_tensor("wqt", [128, KC, DK], BF16, kind="ExternalInput")
    wkt_d = nc.dram_tensor("wkt", [128, KC, DK], BF16, kind="ExternalInput")
    wvt_d = nc.dram_tensor("wvt", [128, KC, DK], BF16, kind="ExternalInput")
    bq_d = nc.dram_tensor("bq", [DK, 1], F32, kind="ExternalInput")
    bk_d = nc.dram_tensor("bk", [DK, 1], F32, kind="ExternalInput")
    wot_d = nc.dram_tensor("wot", [DK, D], BF16, kind="ExternalInput")
    out_d = nc.dram_tensor("out", [D, R], BF16, kind="ExternalOutput")
    with tile.TileContext(nc) as tc:
        _body(tc, nc, xt_d, wqt_d, wkt_d, wvt_d, bq_d, bk_d, wot_d, out_d)
    nc.compile()
    return nc


_NC = None


def _get_nc():
    global _NC
    if _NC is None:
        _NC = build_bass()
    return _NC


def prepare(inputs):
    """Full inputs -> (per-core in_maps, host-side bias constant)."""
    q = np.asarray(inputs["query"], np.float32)
    Wq = np.asarray(inputs["Wq"], np.float32)
    Wk = np.asarray(inputs["Wk"], np.float32)
    Wv = np.asarray(inputs["Wv"], np.float32)
    Wo = np.asarray(inputs["Wo"], np.float32)
    bq = np.asarray(inputs["bq"], np.float32)
    bk = np.asarray(inputs["bk"], np.float32)
    bv = np.asarray(inputs["bv"], np.float32)
    bo = np.asarray(inputs["bo"], np.float32)

    X = q.reshape(R, D)
    # [p, chunk, r] with in-feature = chunk*128 + p
    xt = np.ascontiguousarray(
        X.T.reshape(KC, 128, R).transpose(1, 0, 2)
    ).astype(_BF16_NP)

    def wslice(W, hs):
        # W[hs].T laid out [p, chunk, m]: in-feat within chunk, chunk, out-feat
        return np.ascontiguousarray(
            W[hs, :].T.reshape(KC, 128, DK).transpose(1, 0, 2)
        ).astype(_BF16_NP)

    in_maps = []
    const = bo.astype(np.float64).copy()
    for c in range(N_CORES):
        hs = slice(c * DK, (c + 1) * DK)
        const += Wo[:, hs].astype(np.float64) @ bv[hs].astype(np.float64)
        in_maps.append(
            {
                "xt": xt,
                "wqt": wslice(Wq, hs),
                "wkt": wslice(Wk, hs),
                "wvt": wslice(Wv, hs),
                "bq": np.ascontiguousarray(bq[hs].reshape(DK, 1)),
                "bk": np.ascontiguousarray(bk[hs].reshape(DK, 1)),
                "wot": np.ascontiguousarray(Wo[:, hs].T).astype(_BF16_NP),
            }
        )
    return in_maps, const


def finish(results, const):
    acc = np.zeros((D, R), np.float64)
    for r in results:
        acc += np.asarray(r["out"], np.float64)
    out = acc.T + const[None, :]
    return out.astype(np.float32).reshape(B, L, D)


def run(in_maps, trace=False, **kwargs):
    nc = _get_nc()
    return run_bass_kernel_spmd(nc, in_maps, list(range(N_CORES)), trace=trace, **kwargs)


def kernel(**inputs):
    in_maps, const = prepare(inputs)
    res = run(in_maps)
    return finish(res.results, const)


# revision 13
# speedup vs baseline: 1.1544x; 1.0652x over previous
"""Multi-head attention (B=2, L=2048, D=1024, H=16) on 8 trn2 NeuronCores.

Sharding: tensor-parallel over heads — 2 heads per core. Each core computes
q/k/v projections for its 2 heads, the attention for those heads, and a
row-parallel partial of the output projection (transposed). The host sums
the 8 bf16 partials (the "all-reduce") and adds the biases that were folded
out of the device kernel (bv folded through Wo, plus bo).

Device layout (everything transposed, feature-major):
  xt   [128, KC, R]  : X.T, chunked along the contraction dim
  qt/kt/vt [128, R]  : projections, partitions = 2 heads x 64 head-dims
  va[h][128, NRT,128]: per k-row-tile blocks [v_h | ones] / [ones | v_h];
                       the ones columns make the PV matmul also produce the
                       softmax denominator in the complementary half.
  logitsT [k, q]     : exp() needs no max-subtraction (logits ~ N(0, 0.33^2))

All-bf16 matmul inputs: fp8(+DoubleRow) was measured at rel-err 0.019-0.028
on the final output — over the 2e-2 budget — so it is not used.

Schedule: attention runs in 512-column q units (8 of them). Per k-tile the
two heads' logits land in one [128, 2, 512] PSUM tile (2 banks, bufs=2)
written by a row-group-concurrent MM pair, and ONE ACTIVATE exps both
heads; PV lags by 2 k-tiles so the in-order PE never waits on ACT. ScalarE
(exp) is the pacing engine. The softmax reciprocal is computed as
exp(-ln(d)) so every ACT call lives in the single
natural_log_exp_and_others table set — a raw Reciprocal costs two table
swaps per unit, each ~1.3us plus a >3.4us PE idle that re-throttles the
HAM clock gate to 1.2 GHz.
"""

import numpy as np
import ml_dtypes

import concourse.bass as bass
import concourse.mybir as mybir
import concourse.tile as tile
from concourse import bacc
from concourse.bass_utils import run_bass_kernel_spmd
from concourse.masks import make_identity

# Constrain the ACT table-set chooser to the one set containing BOTH Exp and
# Ln. Left alone, bacc assigns Exp -> exp_and_others and Ln -> natural_log
# (first set containing each func), which costs 2 table reloads per
# attention unit (~1.3us each) plus a >3.4us ScalarE/PE stall that
# re-throttles the PE HAM clock gate to 1.2 GHz for ~3.4us every unit.
# Set IDs are positional indices into act_info.json, so the dict must keep
# all entries (emptied, not removed).
_orig_get_act_tables = bacc.get_activation_tables


def _ln_exp_only_tables(arch):
    t = _orig_get_act_tables(arch)
    return {
        k: (v if k == "natural_log_exp_and_others" else set())
        for k, v in t.items()
    }


bacc.get_activation_tables = _ln_exp_only_tables

B, L, D, H = 2, 2048, 1024, 16
HD = D // H              # 64 head dim
N_CORES = 8
HPC = H // N_CORES       # 2 heads per core
DK = HPC * HD            # 128 local qkv feature dim
R = B * L                # 4096 rows
KC = D // 128            # 8 contraction chunks for the projections
UW = 512                 # attention unit width (q columns)
NUB = L // UW            # 4 units per batch
NRH = R // UW            # 8 projection row-chunk halves
NKT = L // 128           # 16 k tiles per batch
NRT = R // 128           # 32 row tiles
LAG = 2                  # PV k-tile lag behind logits/exp
SCALE = HD ** -0.5

BF16 = mybir.dt.bfloat16
F32 = mybir.dt.float32
Act = mybir.ActivationFunctionType
Alu = mybir.AluOpType

_BF16_NP = ml_dtypes.bfloat16


def _body(tc, nc, xt_d, wqt_d, wkt_d, wvt_d, bq_d, bk_d, wot_d, out_d):
    with (
        tc.tile_pool(name="consts", bufs=1) as constp,
        tc.tile_pool(name="bigs", bufs=1) as bigs,
        tc.tile_pool(name="epool", bufs=4) as epool,
        tc.tile_pool(name="work", bufs=1) as work,
        tc.tile_pool(name="outst", bufs=4) as outst,
    ):
        # ---- load weights / biases ----
        wq_sb = constp.tile([128, KC, DK], BF16)
        wk_sb = constp.tile([128, KC, DK], BF16)
        wv_sb = constp.tile([128, KC, DK], BF16)
        wot_sb = constp.tile([DK, D], BF16)
        bq_sb = constp.tile([DK, 1], F32)
        bk_sb = constp.tile([DK, 1], F32)
        ident = constp.tile([128, 128], BF16)
        nc.sync.dma_start(out=wq_sb, in_=wqt_d[:])
        nc.sync.dma_start(out=wk_sb, in_=wkt_d[:])
        nc.sync.dma_start(out=wv_sb, in_=wvt_d[:])
        nc.sync.dma_start(out=wot_sb, in_=wot_d[:])
        nc.sync.dma_start(out=bq_sb, in_=bq_d[:])
        nc.sync.dma_start(out=bk_sb, in_=bk_d[:])
        make_identity(nc, ident)

        # ---- load X.T (chunked so the projections chase the DMA) ----
        xt_sb = bigs.tile([128, KC, R], BF16)
        for c in range(KC):
            nc.sync.dma_start(out=xt_sb[:, c, :], in_=xt_d[:, c, :])

        qt = bigs.tile([DK, R], BF16)
        kt = bigs.tile([DK, R], BF16)
        vt = bigs.tile([DK, R], BF16)
        yt = bigs.tile([DK, R], BF16)
        va = [bigs.tile([128, NRT, 128], BF16, name=f"va{h}") for h in range(HPC)]
        for h in range(HPC):
            nc.gpsimd.memset(va[h][:], 1.0)

        # ---- q/k/v projections ----
        with tc.tile_pool(name="projpsum", bufs=1, space="PSUM") as projp:
            for wsb, bsb, dest in (
                (wk_sb, bk_sb, kt),
                (wq_sb, bq_sb, qt),
                (wv_sb, None, vt),
            ):
                ps = [
                    projp.tile([128, UW], F32, tag="proj", bufs=NRH, name=f"pp{i}")
                    for i in range(NRH)
                ]
                for c in range(KC):
                    for i in range(NRH):
                        nc.tensor.matmul(
                            ps[i],
                            lhsT=wsb[:, c, :],
                            rhs=xt_sb[:, c, i * UW : (i + 1) * UW],
                            start=(c == 0),
                            stop=(c == KC - 1),
                        )
                for i in range(NRH):
                    cols = slice(i * UW, (i + 1) * UW)
                    if bsb is not None:
                        nc.vector.tensor_scalar_add(
                            out=dest[:, cols], in0=ps[i], scalar1=bsb
                        )
                    else:
                        nc.vector.tensor_copy(out=dest[:, cols], in_=ps[i])

        psum_cm = tc.tile_pool(name="psum", bufs=1, space="PSUM")
        psum = psum_cm.__enter__()

        # ---- va via PE transpose of vt ----
        for t in range(NRT):
            pt = psum.tile([128, 128], BF16, tag="spare", bufs=2, name="pt")
            nc.tensor.transpose(pt, vt[:, t * 128 : (t + 1) * 128], ident)
            for h in range(HPC):
                nc.vector.tensor_copy(
                    out=va[h][:, t, h * HD : (h + 1) * HD],
                    in_=pt[:, h * HD : (h + 1) * HD],
                )

        # ---- attention ----
        def emit_outproj(rc, ofbs):
            # partial outT[ofb-block, unit-cols] = WoTlocal_blk.T @ YT_unit
            for ofb in ofbs:
                po = psum.tile([128, UW], F32, tag="spare", bufs=2, name="po")
                nc.tensor.matmul(
                    po,
                    lhsT=wot_sb[:, ofb * 128 : (ofb + 1) * 128],
                    rhs=yt[:, rc * UW : (rc + 1) * UW],
                    start=True,
                    stop=True,
                )
                ost = outst.tile([128, UW], BF16, name="ost")
                nc.vector.tensor_copy(out=ost, in_=po)
                nc.sync.dma_start(
                    out=out_d[ofb * 128 : (ofb + 1) * 128, rc * UW : (rc + 1) * UW],
                    in_=ost,
                )

        pending_rc = None
        for b in range(B):
            for u in range(NUB):
                qcols = slice(b * L + u * UW, b * L + (u + 1) * UW)
                pv0 = psum.tile([128, UW], F32, tag="pv", bufs=2, name="pv0")
                pv1 = psum.tile([128, UW], F32, tag="pv", bufs=2, name="pv1")
                # software pipeline: PV lags logits/exp by LAG k-tiles so the
                # in-order PE only ever waits on semaphores already satisfied.
                es = {}
                for k in range(NKT + LAG):
                    if k < NKT:
                        kcols = slice(b * L + k * 128, b * L + (k + 1) * 128)
                        # both heads' logits in one 2-bank psum tile; the MM
                        # pair targets disjoint PE row groups (auto
                        # tile_position from base_partition 0 / 64).
                        pl = psum.tile([128, HPC, UW], F32, tag="pl", bufs=2, name="pl")
                        nc.tensor.matmul(
                            pl[:, 0, :], lhsT=kt[0:HD, kcols], rhs=qt[0:HD, qcols],
                            start=True, stop=True,
                        )
                        nc.tensor.matmul(
                            pl[:, 1, :], lhsT=kt[HD:DK, kcols], rhs=qt[HD:DK, qcols],
                            start=True, stop=True,
                        )
                        e = epool.tile([128, HPC, UW], BF16, name="e")
                        nc.scalar.activation(
                            out=e[:, :, :], in_=pl[:, :, :], func=Act.Exp, scale=SCALE
                        )
                        es[k] = e
                    if k >= LAG:
                        j = k - LAG
                        tg = b * NKT + j
                        ep = es.pop(j)
                        nc.tensor.matmul(
                            pv0, lhsT=va[0][:, tg, :], rhs=ep[:, 0, :],
                            start=(j == 0), stop=(j == NKT - 1),
                        )
                        nc.tensor.matmul(
                            pv1, lhsT=va[1][:, tg, :], rhs=ep[:, 1, :],
                            start=(j == 0), stop=(j == NKT - 1),
                        )
                    # previous unit's out-projection, spread 1 block/ktile
                    if pending_rc is not None and 3 <= k <= 10:
                        emit_outproj(pending_rc, [k - 3])
                        if k == 10:
                            pending_rc = None
                # pv0 = [Yun_h0 (p 0:64); denom_h0 (p 64:128)]
                # pv1 = [denom_h1 (p 0:64); Yun_h1 (p 64:128)]
                # Unit tail, ordered so nothing gates the next unit: ACT does
                # ln(d) then exp(-x) back to back (reciprocal in the exp table
                # set, BEFORE the cross-partition swap so the strict-FIFO ACT
                # queue never waits on a DMA); DVE evacuates Y so the pv psum
                # banks free up before the next unit's first PV matmul.
                rsw = work.tile([128, UW], F32, tag="rsw", bufs=2, name="rsw")
                nc.scalar.activation(out=rsw[HD:128, :], in_=pv0[HD:128, :], func=Act.Ln)
                nc.scalar.activation(out=rsw[0:HD, :], in_=pv1[0:HD, :], func=Act.Ln)
                ysb = work.tile([128, UW], BF16, tag="ysb", bufs=2, name="ysb")
                nc.vector.tensor_copy(out=ysb[0:HD, :], in_=pv0[0:HD, :])
                nc.vector.tensor_copy(out=ysb[HD:128, :], in_=pv1[HD:128, :])
                rre = work.tile([128, UW], F32, tag="rre", bufs=2, name="rre")
                nc.scalar.activation(out=rre, in_=rsw, func=Act.Exp, scale=-1.0)
                # swap halves across partitions (DMA is the cross-lane engine)
                rrs = work.tile([128, UW], F32, tag="rrs", bufs=2, name="rrs")
                nc.sync.dma_start(out=rrs[0:HD, :], in_=rre[HD:128, :])
                nc.sync.dma_start(out=rrs[HD:128, :], in_=rre[0:HD, :])
                nc.vector.tensor_mul(
                    out=yt[0:HD, qcols], in0=ysb[0:HD, :], in1=rrs[0:HD, :]
                )
                nc.vector.tensor_mul(
                    out=yt[HD:DK, qcols], in0=ysb[HD:DK, :], in1=rrs[HD:DK, :]
                )
                pending_rc = b * NUB + u

        # ---- last unit's out-projection ----
        emit_outproj(pending_rc, range(D // 128))
        psum_cm.__exit__(None, None, None)


def build_bass():
    nc = bacc.Bacc("TRN2", target_bir_lowering=False, debug=False)
    xt_d = nc.dram_tensor("xt", [128, KC, R], BF16, kind="ExternalInput")
    wqt_d = nc.dram_tensor("wqt", [128, KC, DK], BF16, kind="ExternalInput")
    wkt_d = nc.dram_tensor("wkt", [128, KC, DK], BF16, kind="ExternalInput")
    wvt_d = nc.dram_tensor("wvt", [128, KC, DK], BF16, kind="ExternalInput")
    bq_d = nc.dram_tensor("bq", [DK, 1], F32, kind="ExternalInput")
    bk_d = nc.dram_tensor("bk", [DK, 1], F32, kind="ExternalInput")
    wot_d = nc.dram_tensor("wot", [DK, D], BF16, kind="ExternalInput")
    out_d = nc.dram_tensor("out", [D, R], BF16, kind="ExternalOutput")
    with tile.TileContext(nc) as tc:
        _body(tc, nc, xt_d, wqt_d, wkt_d, wvt_d, bq_d, bk_d, wot_d, out_d)
    nc.compile()
    return nc


_NC = None


def _get_nc():
    global _NC
    if _NC is None:
        _NC = build_bass()
    return _NC


def prepare(inputs):
    """Full inputs -> (per-core in_maps, host-side bias constant)."""
    q = np.asarray(inputs["query"], np.float32)
    Wq = np.asarray(inputs["Wq"], np.float32)
    Wk = np.asarray(inputs["Wk"], np.float32)
    Wv = np.asarray(inputs["Wv"], np.float32)
    Wo = np.asarray(inputs["Wo"], np.float32)
    bq = np.asarray(inputs["bq"], np.float32)
    bk = np.asarray(inputs["bk"], np.float32)
    bv = np.asarray(inputs["bv"], np.float32)
    bo = np.asarray(inputs["bo"], np.float32)

    X = q.reshape(R, D)
    # [p, chunk, r] with in-feature = chunk*128 + p
    xt = np.ascontiguousarray(
        X.T.reshape(KC, 128, R).transpose(1, 0, 2)
    ).astype(_BF16_NP)

    def wslice(W, hs):
        # W[hs].T laid out [p, chunk, m]: in-feat within chunk, chunk, out-feat
        return np.ascontiguousarray(
            W[hs, :].T.reshape(KC, 128, DK).transpose(1, 0, 2)
        ).astype(_BF16_NP)

    in_maps = []
    const = bo.astype(np.float64).copy()
    for c in range(N_CORES):
        hs = slice(c * DK, (c + 1) * DK)
        const += Wo[:, hs].astype(np.float64) @ bv[hs].astype(np.float64)
        in_maps.append(
            {
                "xt": xt,
                "wqt": wslice(Wq, hs),
                "wkt": wslice(Wk, hs),
                "wvt": wslice(Wv, hs),
                "bq": np.ascontiguousarray(bq[hs].reshape(DK, 1)),
                "bk": np.ascontiguousarray(bk[hs].reshape(DK, 1)),
                "wot": np.ascontiguousarray(Wo[:, hs].T).astype(_BF16_NP),
            }
        )
    return in_maps, const


def finish(results, const):
    acc = np.zeros((D, R), np.float64)
    for r in results:
        acc += np.asarray(r["out"], np.float64)
    out = acc.T + const[None, :]
    return out.astype(np.float32).reshape(B, L, D)


def run(in_maps, trace=False, **kwargs):
    nc = _get_nc()
    return run_bass_kernel_spmd(nc, in_maps, list(range(N_CORES)), trace=trace, **kwargs)


def kernel(**inputs):
    in_maps, const = prepare(inputs)
    res = run(in_maps)
    return finish(res.results, const)
